# revision 1
# baseline (speedup 1.0000x reference)
"""GridToStation Trainium2 kernel.

Strategy (sharding_hint: shard grid over nlon, route stations to owning shard):
  - Host: transpose grid (C,H,W) -> (H,W,C); shard over W into 8 shards of 180
    columns + 1 halo column (duplicated edge for the last shard). Each shard is
    a gather table of shape (721*181, 256) f32 whose rows are grid points.
  - Host: compute per-station bilinear corner indices + weights exactly as the
    reference does (f32 math), bucket stations by owning shard, pad each
    bucket to a common padded count.
  - Device (per core, SPMD over 8 cores):
      * indirect (gather) DMA pulls, for each station, two 2KB rows:
        [v00|v01] at (iy0, ix0..ix0+1) and [v10|v11] at (iy1, ix0..ix0+1).
      * bilinear combine fused into PE transpose: for each corner j,
        matmul(out_psum += v_j_chunk^T @ diag(c_j)) accumulates the weighted
        transpose x^T [C, stations] directly in PSUM.
      * MLP: h = gelu(x @ W1^T + b1); y = h @ W2^T + b2 computed in
        [C, stations] layout (bias is then per-partition, fed to ACT).
      * One big output write at the end: y^T stored as [256, n_pad] in DRAM.
  - Host: gather per-core outputs, inverse-permute to original station order.
"""

import os

import numpy as np

B, C, H, W, N = 1, 256, 721, 1440, 16384
NCORES = 8
COLS = W // NCORES  # 180 owned columns per shard
WP = COLS + 1  # +1 halo column
TROWS = H * WP  # gather-table rows per shard
R = 4  # station tiles (of 128) per gather group
PAIR_T = 2  # station tiles per MLP batch (N=256)

# matmul operand dtype knob: "f32" (exact) or "f32r" (fast, HW-reduced precision)
MM_DTYPE = os.environ.get("GRIDSTN_MM_DTYPE", "f32")

_PROG_CACHE = {}


def _f32(x):
    return np.float32(x)


def _host_route(station_coords):
    """Replicate the reference index math in f32, bucket stations by shard."""
    lat = np.asarray(station_coords[0, :, 0], dtype=np.float32)
    lon = np.asarray(station_coords[0, :, 1], dtype=np.float32)
    lat_n = lat / _f32(90.0)
    lon_n = lon / _f32(180.0)
    ix = np.clip((lon_n + _f32(1.0)) * _f32(0.5) * _f32(W - 1), _f32(0.0), _f32(W - 1))
    iy = np.clip((lat_n + _f32(1.0)) * _f32(0.5) * _f32(H - 1), _f32(0.0), _f32(H - 1))
    ix0f = np.floor(ix)
    iy0f = np.floor(iy)
    wx = (ix - ix0f).astype(np.float32)
    wy = (iy - iy0f).astype(np.float32)
    ix0 = ix0f.astype(np.int32)
    iy0 = iy0f.astype(np.int32)
    iy1 = np.minimum(iy0 + 1, H - 1)
    owner = ix0 // COLS  # ix0 <= 1439 -> owner <= 7
    x0l = ix0 - owner * COLS  # 0..179; +1 stays inside WP=181
    row0 = iy0 * WP + x0l
    row1 = iy1 * WP + x0l
    one = _f32(1.0)
    c00 = (one - wx) * (one - wy)
    c01 = wx * (one - wy)
    c10 = (one - wx) * wy
    c11 = wx * wy
    return owner, row0, row1, (c00, c01, c10, c11)


def _host_tables(grid_features):
    g = np.asarray(grid_features[0], dtype=np.float32)  # (C, H, W)
    gt = np.ascontiguousarray(np.transpose(g, (1, 2, 0)))  # (H, W, C)
    tables = []
    for c in range(NCORES):
        lo = c * COLS
        if c < NCORES - 1:
            tbl = gt[:, lo : lo + WP, :]
        else:
            tbl = np.concatenate([gt[:, lo:W, :], gt[:, W - 1 : W, :]], axis=1)
        tables.append(np.ascontiguousarray(tbl).reshape(TROWS, C))
    return tables


def _build_program(G):
    import concourse.bacc as bacc
    import concourse.bass as bass
    import concourse.mybir as mybir
    from concourse.tile import TileContext

    f32 = mybir.dt.float32
    i32 = mybir.dt.int32
    mm_dt = mybir.dt.float32r if MM_DTYPE == "f32r" else f32

    T = G * R  # station tiles total
    NP = T * 128  # padded stations per core

    nc = bacc.Bacc("TRN2", target_bir_lowering=False, debug=False)

    tbl = nc.dram_tensor("tbl", [TROWS, C], f32, kind="ExternalInput")
    idx = nc.dram_tensor("idx", [128, G * 2 * R], i32, kind="ExternalInput")
    cof = nc.dram_tensor("cof", [128, 4 * T], f32, kind="ExternalInput")
    w1 = nc.dram_tensor("w1t", [C, C], f32, kind="ExternalInput")
    w2 = nc.dram_tensor("w2t", [C, C], f32, kind="ExternalInput")
    bia = nc.dram_tensor("bia", [128, 4], f32, kind="ExternalInput")
    idn = nc.dram_tensor("idn", [128, 128], f32, kind="ExternalInput")
    out = nc.dram_tensor("out", [2, 128, NP], f32, kind="ExternalOutput")

    def mm(ap):
        return ap.bitcast(mm_dt) if mm_dt != f32 else ap

    with TileContext(nc) as tc:
        with (
            tc.tile_pool(name="const", bufs=1) as cpool,
            tc.tile_pool(name="gat", bufs=3) as gpool,
            tc.tile_pool(name="sm", bufs=4) as spool,
            tc.tile_pool(name="xs", bufs=4) as xpool,
            tc.tile_pool(name="hs", bufs=4) as hpool,
            tc.tile_pool(name="px", bufs=2, space="PSUM") as pxp,
            tc.tile_pool(name="ph", bufs=1, space="PSUM") as php,
            tc.tile_pool(name="py", bufs=1, space="PSUM") as pyp,
        ):
            idx_sb = cpool.tile([128, G * 2 * R], i32)
            nc.sync.dma_start(out=idx_sb[:], in_=idx[:])
            cof_sb = cpool.tile([128, 4 * T], f32)
            nc.sync.dma_start(out=cof_sb[:], in_=cof[:])
            w1_sb = cpool.tile([128, 2 * C], f32)
            nc.sync.dma_start(out=w1_sb[:, 0:C], in_=w1[0:128, :])
            nc.sync.dma_start(out=w1_sb[:, C : 2 * C], in_=w1[128:256, :])
            w2_sb = cpool.tile([128, 2 * C], f32)
            nc.sync.dma_start(out=w2_sb[:, 0:C], in_=w2[0:128, :])
            nc.sync.dma_start(out=w2_sb[:, C : 2 * C], in_=w2[128:256, :])
            bia_sb = cpool.tile([128, 4], f32)
            nc.sync.dma_start(out=bia_sb[:], in_=bia[:])
            idn_sb = cpool.tile([128, 128], f32)
            nc.sync.dma_start(out=idn_sb[:], in_=idn[:])
            out_sb = cpool.tile([128, 2 * NP], f32)

            for gi in range(G):
                gt_t = gpool.tile([128, 2 * R * 512], f32)
                # HW indirect DMA honors one dynamic offset per partition:
                # issue one gather per 512-elem block (rows idx, idx+1).
                for q in range(2 * R):
                    nc.gpsimd.indirect_dma_start(
                        out=gt_t[:, q * 512 : (q + 1) * 512],
                        out_offset=None,
                        in_=tbl[:],
                        in_offset=bass.IndirectOffsetOnAxis(
                            ap=idx_sb[:, gi * 2 * R + q : gi * 2 * R + q + 1], axis=0
                        ),
                    )
                for pr in range(R // PAIR_T):
                    pxs = [pxp.tile([128, 256], f32, name=f"px{i}") for i in range(2)]
                    for tt in range(PAIR_T):
                        r = pr * PAIR_T + tt
                        tg = gi * R + r
                        # bilinear combine on DVE in [station, C] layout:
                        # acc = v00*c00; acc = vj*cj + acc (fused) x3
                        sm = spool.tile([128, 256], f32, name=f"sm{tt}")
                        for j in range(4):
                            y = j >> 1
                            xo = (j & 1) * 256
                            base = (y * R + r) * 512 + xo
                            vj = gt_t[:, base : base + 256]
                            cj = cof_sb[:, j * T + tg : j * T + tg + 1]
                            if j == 0:
                                # seed on ScalarE (has slack): sm = vj * cj
                                nc.scalar.activation(
                                    out=sm[:],
                                    in_=vj,
                                    func=mybir.ActivationFunctionType.Copy,
                                    scale=cj,
                                )
                            else:
                                nc.vector.scalar_tensor_tensor(
                                    out=sm[:],
                                    in0=vj,
                                    scalar=cj,
                                    in1=sm[:],
                                    op0=mybir.AluOpType.mult,
                                    op1=mybir.AluOpType.add,
                                )
                        # plain permutation transpose to [C, station] in PSUM
                        for ch in range(2):
                            nc.tensor.transpose(
                                out=pxs[ch][:, tt * 128 : (tt + 1) * 128],
                                in_=sm[:, ch * 128 : (ch + 1) * 128],
                                identity=idn_sb[:],
                            )
                    xss = [xpool.tile([128, 256], f32, name=f"xs{i}") for i in range(2)]
                    for ch in range(2):
                        nc.scalar.activation(
                            out=xss[ch][:],
                            in_=pxs[ch][:],
                            func=mybir.ActivationFunctionType.Copy,
                        )
                    phs = [php.tile([128, 256], f32, name=f"ph{i}") for i in range(2)]
                    for m in range(2):
                        for k in range(2):
                            nc.tensor.matmul(
                                out=phs[m][:],
                                lhsT=mm(w1_sb[:, k * C + m * 128 : k * C + (m + 1) * 128]),
                                rhs=mm(xss[k][:]),
                                start=(k == 0),
                                stop=(k == 1),
                            )
                    hss = [hpool.tile([128, 256], f32, name=f"hs{i}") for i in range(2)]
                    for m in range(2):
                        nc.scalar.activation(
                            out=hss[m][:],
                            in_=phs[m][:],
                            func=mybir.ActivationFunctionType.Gelu,
                            bias=bia_sb[:, m : m + 1],
                            scale=1.0,
                        )
                    pys = [pyp.tile([128, 256], f32, name=f"py{i}") for i in range(2)]
                    for m in range(2):
                        for k in range(2):
                            nc.tensor.matmul(
                                out=pys[m][:],
                                lhsT=mm(w2_sb[:, k * C + m * 128 : k * C + (m + 1) * 128]),
                                rhs=mm(hss[k][:]),
                                start=(k == 0),
                                stop=(k == 1),
                            )
                    col = (gi * R + pr * PAIR_T) * 128
                    for m in range(2):
                        nc.vector.tensor_scalar_add(
                            out_sb[:, m * NP + col : m * NP + col + 256],
                            pys[m][:],
                            bia_sb[:, 2 + m : 3 + m],
                        )
            nc.sync.dma_start(out=out[0], in_=out_sb[:, 0:NP])
            nc.sync.dma_start(out=out[1], in_=out_sb[:, NP : 2 * NP])
    return nc, NP


def _make_in_maps(grid_features, station_coords, W1, b1, W2, b2):
    owner, row0, row1, cjs = _host_route(station_coords)
    tables = _host_tables(grid_features)

    sids_per_core = [np.nonzero(owner == c)[0] for c in range(NCORES)]
    max_n = max(len(s) for s in sids_per_core)
    G = max(1, -(-max_n // (R * 128)))  # ceil
    T = G * R
    NP = T * 128

    w1t = np.ascontiguousarray(np.asarray(W1, np.float32).T)
    w2t = np.ascontiguousarray(np.asarray(W2, np.float32).T)
    bia = np.zeros((128, 4), np.float32)
    bia[:, 0] = b1[0:128]
    bia[:, 1] = b1[128:256]
    bia[:, 2] = b2[0:128]
    bia[:, 3] = b2[128:256]
    idn = np.eye(128, dtype=np.float32)

    in_maps = []
    for c in range(NCORES):
        sids = sids_per_core[c]
        nl = len(sids)
        r0 = np.zeros(NP, np.int32)
        r1 = np.zeros(NP, np.int32)
        r0[:nl] = row0[sids]
        r1[:nl] = row1[sids]
        cj = np.zeros((4, NP), np.float32)
        for j in range(4):
            cj[j, :nl] = cjs[j][sids]
        r0m = r0.reshape(T, 128).T  # [128, T]
        r1m = r1.reshape(T, 128).T
        idx_arr = np.zeros((128, G * 2 * R), np.int32)
        for g in range(G):
            idx_arr[:, g * 2 * R : g * 2 * R + R] = r0m[:, g * R : (g + 1) * R]
            idx_arr[:, g * 2 * R + R : (g + 1) * 2 * R] = r1m[:, g * R : (g + 1) * R]
        cof_arr = np.ascontiguousarray(
            np.concatenate([cj[j].reshape(T, 128).T for j in range(4)], axis=1)
        )
        in_maps.append(
            {
                "tbl": tables[c],
                "idx": np.ascontiguousarray(idx_arr),
                "cof": cof_arr,
                "w1t": w1t,
                "w2t": w2t,
                "bia": bia,
                "idn": idn,
            }
        )
    return in_maps, sids_per_core, G, NP


LAST_RUN_INFO = {}


def _install_ntff_shim():
    """This container's antenv lacks axon_hooks; provide the NTFF profile
    hook via the same ctypes path trn_boot would have used."""
    import sys
    import types

    try:
        import antenv.axon_hooks  # noqa: F401

        return
    except ImportError:
        pass
    from trn_agent_boot.trn_boot import _ntff_profile_via_ctypes

    hook = _ntff_profile_via_ctypes("/opt/axon/libaxon_pjrt.so")
    mod = types.ModuleType("antenv.axon_hooks")
    mod.get_axon_ntff_profile_hook = lambda: hook
    mod.set_axon_ntff_profile_hook = lambda h: None
    sys.modules["antenv.axon_hooks"] = mod


def kernel(grid_features, station_coords, W1, b1, W2, b2):
    in_maps, sids_per_core, G, NP = _make_in_maps(
        grid_features, station_coords, W1, b1, W2, b2
    )

    key = (G, MM_DTYPE)
    if key not in _PROG_CACHE:
        _PROG_CACHE[key] = _build_program(G)
    nc, NP2 = _PROG_CACHE[key]
    assert NP2 == NP

    if os.environ.get("GRIDSTN_SIM"):
        outs = _run_sim(nc, in_maps)
    else:
        from concourse.bass_utils import run_bass_kernel_spmd

        trace = bool(os.environ.get("GRIDSTN_TRACE"))
        if trace:
            _install_ntff_shim()
        if not nc.is_finalized():
            nc.finalize()
        res = run_bass_kernel_spmd(
            nc, in_maps, list(range(NCORES)), trace=trace
        )
        LAST_RUN_INFO["exec_time_ns"] = res.exec_time_ns
        LAST_RUN_INFO["mean_exec_time_ns"] = res.mean_exec_time_ns
        LAST_RUN_INFO["profile_json"] = res.profile_json
        outs = [r["out"] for r in res.results]

    result = np.zeros((N, C), np.float32)
    for c in range(NCORES):
        sids = sids_per_core[c]
        y = outs[c].reshape(2 * 128, NP)
        result[sids] = y[:, : len(sids)].T
    return result.reshape(B, N, C)


def _run_sim(nc, in_maps):
    from concourse import bass_interp

    outs = []
    for c in range(NCORES):
        sim = bass_interp.MultiCoreSim(nc, 1)
        for name, arr in in_maps[c].items():
            sim.cores[0].tensor(name)[:] = arr
        sim.simulate()
        LAST_RUN_INFO["sim_time_ns"] = sim.cores[0].time
        outs.append(np.array(sim.cores[0].tensor("out")))
        if os.environ.get("GRIDSTN_SIM_ONE_CORE"):
            # replicate core 0's output for the rest (fast smoke mode)
            outs = outs + [outs[0]] * (NCORES - 1)
            break
    return outs



# revision 5
# speedup vs baseline: 1.5243x; 1.5243x over previous
"""GridToStation Trainium2 kernel (bf16 patch-gather version).

Strategy:
  - Host: transpose grid (C,H,W) -> (H,W,C), cast bf16, and build per-core
    2x2 PATCH tables: row (y, xl) = [g[y,x], g[y,x1], g[y1,x], g[y1,x1]]
    (1024 bf16 = 2KB), where each core's table is a 212-column lon window
    around its 180-column band (x1/y1 border-clipped at build time).
  - Host: replicate the reference's f32 index math; sort stations by ix0 and
    cut every N/8 -> EXACTLY 2048 stations per core (fits each core's window
    because station lon is near-uniform; asserted). No padding anywhere.
  - Device (per core, SPMD over 8 cores), 16 tiles of 128 stations:
      * one indirect (gather) DMA per tile pulls 128 patch rows (2KB each).
        One descriptor per station is the minimum serial GpSimd/SWDGE work
        (~1.1us per call fixed) -- this is the kernel's critical path.
      * bilinear combine in [station, C] bf16: ACT seed (scale=c00) + 3 DVE
        scalar_tensor_tensor fused multiply-adds.
      * PE transpose (bf16) -> PSUM f32 -> ACT copy to bf16 x^T tiles.
      * MLP in [C, station] layout, groups of 512 stations: W1/W2 bf16
        matmuls (PSUM f32), GELU+b1 on ACT, b2 add on DVE, bf16 out.
      * per-group output DMA (overlapped), 2x[128,512] bf16 per group.
  - Host: upcast bf16 -> f32 and inverse-permute to original station order.
"""

import os

import numpy as np
import ml_dtypes

B, C, H, W, N = 1, 256, 721, 1440, 16384
NCORES = 8
COLS = W // NCORES  # 180 owned columns per core
FX = 16  # window flex columns each side
WT = COLS + 2 * FX  # 212-column table window
TROWS = H * WT  # patch-table rows per core
NP = N // NCORES  # 2048 stations per core, exact
T = NP // 128  # 16 tiles
GRP = 4  # tiles per MLP group (512 stations)
NG = T // GRP

_PROG_CACHE = {}


def _f32(x):
    return np.float32(x)


def _host_route(station_coords):
    """Replicate the reference index math in f32."""
    lat = np.asarray(station_coords[0, :, 0], dtype=np.float32)
    lon = np.asarray(station_coords[0, :, 1], dtype=np.float32)
    lat_n = lat / _f32(90.0)
    lon_n = lon / _f32(180.0)
    ix = np.clip((lon_n + _f32(1.0)) * _f32(0.5) * _f32(W - 1), _f32(0.0), _f32(W - 1))
    iy = np.clip((lat_n + _f32(1.0)) * _f32(0.5) * _f32(H - 1), _f32(0.0), _f32(H - 1))
    ix0f = np.floor(ix)
    iy0f = np.floor(iy)
    wx = (ix - ix0f).astype(np.float32)
    wy = (iy - iy0f).astype(np.float32)
    ix0 = ix0f.astype(np.int64)
    iy0 = iy0f.astype(np.int64)
    one = _f32(1.0)
    c00 = (one - wx) * (one - wy)
    c01 = wx * (one - wy)
    c10 = (one - wx) * wy
    c11 = wx * wy
    return ix0, iy0, (c00, c01, c10, c11)


def _host_tables(grid_features):
    """Global (H, W, C) bf16 grid + per-core patch-table windows."""
    g = np.asarray(grid_features[0], dtype=np.float32)  # (C, H, W)
    gt = np.transpose(g, (1, 2, 0)).astype(ml_dtypes.bfloat16)  # (H, W, C)
    # x+1 / y+1 with border clip
    gx1 = np.concatenate([gt[:, 1:, :], gt[:, W - 1 : W, :]], axis=1)
    gy1 = np.concatenate([gt[1:, :, :], gt[H - 1 : H, :, :]], axis=0)
    gx1y1 = np.concatenate([gy1[:, 1:, :], gy1[:, W - 1 : W, :]], axis=1)
    los = [min(max(c * COLS - FX, 0), W - WT) for c in range(NCORES)]
    tables = []
    for c in range(NCORES):
        lo = los[c]
        p = np.empty((H, WT, 4 * C), dtype=ml_dtypes.bfloat16)
        p[:, :, 0:C] = gt[:, lo : lo + WT]
        p[:, :, C : 2 * C] = gx1[:, lo : lo + WT]
        p[:, :, 2 * C : 3 * C] = gy1[:, lo : lo + WT]
        p[:, :, 3 * C : 4 * C] = gx1y1[:, lo : lo + WT]
        tables.append(p.reshape(TROWS, 4 * C))
    return tables, los


def _build_program():
    import concourse.bacc as bacc
    import concourse.bass as bass
    import concourse.mybir as mybir
    from concourse.tile import TileContext

    f32 = mybir.dt.float32
    bf16 = mybir.dt.bfloat16
    i32 = mybir.dt.int32

    nc = bacc.Bacc("TRN2", target_bir_lowering=False, debug=False)

    tbl = nc.dram_tensor("tbl", [TROWS, 4 * C], bf16, kind="ExternalInput")
    idx = nc.dram_tensor("idx", [128, T], i32, kind="ExternalInput")
    cof = nc.dram_tensor("cof", [128, 4 * T], f32, kind="ExternalInput")
    w1 = nc.dram_tensor("w1t", [C, C], bf16, kind="ExternalInput")
    w2 = nc.dram_tensor("w2t", [C, C], bf16, kind="ExternalInput")
    bia = nc.dram_tensor("bia", [128, 4], f32, kind="ExternalInput")
    idn = nc.dram_tensor("idn", [128, 128], bf16, kind="ExternalInput")
    out = nc.dram_tensor("out", [2, 128, NP], bf16, kind="ExternalOutput")

    with TileContext(nc) as tc:
        with (
            tc.tile_pool(name="const", bufs=1) as cpool,
            tc.tile_pool(name="gat", bufs=1) as gpool,
            tc.tile_pool(name="sm", bufs=2 * GRP) as spool,
            tc.tile_pool(name="xs", bufs=2) as xpool,
            tc.tile_pool(name="hs", bufs=2) as hpool,
            tc.tile_pool(name="ys", bufs=2) as ypool,
            tc.tile_pool(name="px", bufs=2, space="PSUM") as pxp,
            tc.tile_pool(name="ph", bufs=1, space="PSUM") as php,
            tc.tile_pool(name="py", bufs=1, space="PSUM") as pyp,
        ):
            idx_sb = cpool.tile([128, T], i32)
            nc.sync.dma_start(out=idx_sb[:], in_=idx[:])
            cof_sb = cpool.tile([128, 4 * T], f32)
            nc.sync.dma_start(out=cof_sb[:], in_=cof[:])
            w1_sb = cpool.tile([128, 2 * C], bf16)
            nc.sync.dma_start(out=w1_sb[:, 0:C], in_=w1[0:128, :])
            nc.sync.dma_start(out=w1_sb[:, C : 2 * C], in_=w1[128:256, :])
            w2_sb = cpool.tile([128, 2 * C], bf16)
            nc.sync.dma_start(out=w2_sb[:, 0:C], in_=w2[0:128, :])
            nc.sync.dma_start(out=w2_sb[:, C : 2 * C], in_=w2[128:256, :])
            bia_sb = cpool.tile([128, 4], f32)
            nc.sync.dma_start(out=bia_sb[:], in_=bia[:])
            idn_sb = cpool.tile([128, 128], bf16)
            nc.sync.dma_start(out=idn_sb[:], in_=idn[:])

            # all gathers issued up-front; enough buffers that the GpSimd
            # queue never stalls (this is the serial critical path)
            gts = []
            for t in range(T):
                gt_t = gpool.tile([128, 1024], bf16, name=f"gt{t}")
                nc.gpsimd.indirect_dma_start(
                    out=gt_t[:],
                    out_offset=None,
                    in_=tbl[:],
                    in_offset=bass.IndirectOffsetOnAxis(
                        ap=idx_sb[:, t : t + 1], axis=0
                    ),
                )
                gts.append(gt_t)

            for g in range(NG):
                pxs = [pxp.tile([128, 512], bf16, name=f"px{c}") for c in range(2)]
                for tt in range(GRP):
                    t = g * GRP + tt
                    gt_t = gts[t]
                    sm = spool.tile([128, 256], bf16, name="sm")
                    for j in range(4):
                        vj = gt_t[:, j * 256 : (j + 1) * 256]
                        cj = cof_sb[:, j * T + t : j * T + t + 1]
                        if j == 0:
                            nc.scalar.activation(
                                out=sm[:],
                                in_=vj,
                                func=mybir.ActivationFunctionType.Copy,
                                scale=cj,
                            )
                        else:
                            nc.vector.scalar_tensor_tensor(
                                out=sm[:],
                                in0=vj,
                                scalar=cj,
                                in1=sm[:],
                                op0=mybir.AluOpType.mult,
                                op1=mybir.AluOpType.add,
                            )
                    for ch in range(2):
                        nc.tensor.transpose(
                            out=pxs[ch][:, tt * 128 : (tt + 1) * 128],
                            in_=sm[:, ch * 128 : (ch + 1) * 128],
                            identity=idn_sb[:],
                        )
                xss = [xpool.tile([128, 512], bf16, name=f"xs{c}") for c in range(2)]
                for ch in range(2):
                    nc.scalar.activation(
                        out=xss[ch][:],
                        in_=pxs[ch][:],
                        func=mybir.ActivationFunctionType.Copy,
                    )
                ph = php.tile([128, 1024], f32, name="ph")
                for m in range(2):
                    for k in range(2):
                        nc.tensor.matmul(
                            out=ph[:, m * 512 : (m + 1) * 512],
                            lhsT=w1_sb[:, k * C + m * 128 : k * C + (m + 1) * 128],
                            rhs=xss[k][:],
                            start=(k == 0),
                            stop=(k == 1),
                        )
                hss = [hpool.tile([128, 512], bf16, name=f"hs{c}") for c in range(2)]
                for m in range(2):
                    nc.scalar.activation(
                        out=hss[m][:],
                        in_=ph[:, m * 512 : (m + 1) * 512],
                        func=mybir.ActivationFunctionType.Gelu,
                        bias=bia_sb[:, m : m + 1],
                        scale=1.0,
                    )
                py = pyp.tile([128, 1024], f32, name="py")
                for m in range(2):
                    for k in range(2):
                        nc.tensor.matmul(
                            out=py[:, m * 512 : (m + 1) * 512],
                            lhsT=w2_sb[:, k * C + m * 128 : k * C + (m + 1) * 128],
                            rhs=hss[k][:],
                            start=(k == 0),
                            stop=(k == 1),
                        )
                yss = [ypool.tile([128, 512], bf16, name=f"ys{c}") for c in range(2)]
                for m in range(2):
                    nc.vector.tensor_scalar_add(
                        yss[m][:],
                        py[:, m * 512 : (m + 1) * 512],
                        bia_sb[:, 2 + m : 3 + m],
                    )
                    nc.sync.dma_start(
                        out=out[m][:, g * 512 : (g + 1) * 512], in_=yss[m][:]
                    )
    return nc


def _make_in_maps(grid_features, station_coords, W1, b1, W2, b2):
    ix0, iy0, cjs = _host_route(station_coords)
    tables, los = _host_tables(grid_features)

    order = np.argsort(ix0, kind="stable")
    w1t = np.ascontiguousarray(np.asarray(W1, np.float32).T).astype(ml_dtypes.bfloat16)
    w2t = np.ascontiguousarray(np.asarray(W2, np.float32).T).astype(ml_dtypes.bfloat16)
    bia = np.zeros((128, 4), np.float32)
    bia[:, 0] = b1[0:128]
    bia[:, 1] = b1[128:256]
    bia[:, 2] = b2[0:128]
    bia[:, 3] = b2[128:256]
    idn = np.eye(128, dtype=ml_dtypes.bfloat16)

    in_maps = []
    sids_per_core = []
    for c in range(NCORES):
        sids = order[c * NP : (c + 1) * NP]
        sids_per_core.append(sids)
        xl = ix0[sids] - los[c]
        assert xl.min() >= 0 and xl.max() < WT, (
            f"core {c}: station lon outside table window "
            f"({xl.min()}..{xl.max()} vs 0..{WT - 1})"
        )
        rows = (iy0[sids] * WT + xl).astype(np.int32)
        idx_arr = np.ascontiguousarray(rows.reshape(T, 128).T)  # [128, T]
        cof_arr = np.ascontiguousarray(
            np.concatenate(
                [cjs[j][sids].reshape(T, 128).T for j in range(4)], axis=1
            )
        ).astype(np.float32)
        in_maps.append(
            {
                "tbl": tables[c],
                "idx": idx_arr,
                "cof": cof_arr,
                "w1t": w1t,
                "w2t": w2t,
                "bia": bia,
                "idn": idn,
            }
        )
    return in_maps, sids_per_core


LAST_RUN_INFO = {}


def _install_ntff_shim():
    """This container's antenv lacks axon_hooks; provide the NTFF profile
    hook via the same ctypes path trn_boot would have used."""
    import sys
    import types

    try:
        import antenv.axon_hooks  # noqa: F401

        return
    except ImportError:
        pass
    from trn_agent_boot.trn_boot import _ntff_profile_via_ctypes

    hook = _ntff_profile_via_ctypes("/opt/axon/libaxon_pjrt.so")
    mod = types.ModuleType("antenv.axon_hooks")
    mod.get_axon_ntff_profile_hook = lambda: hook
    mod.set_axon_ntff_profile_hook = lambda h: None
    sys.modules["antenv.axon_hooks"] = mod


def kernel(grid_features, station_coords, W1, b1, W2, b2):
    in_maps, sids_per_core = _make_in_maps(
        grid_features, station_coords, W1, b1, W2, b2
    )

    if "prog" not in _PROG_CACHE:
        _PROG_CACHE["prog"] = _build_program()
    nc = _PROG_CACHE["prog"]

    if os.environ.get("GRIDSTN_SIM"):
        outs = _run_sim(nc, in_maps)
    else:
        from concourse.bass_utils import run_bass_kernel_spmd

        trace = bool(os.environ.get("GRIDSTN_TRACE"))
        if trace:
            _install_ntff_shim()
        if not nc.is_finalized():
            nc.finalize()
        res = run_bass_kernel_spmd(nc, in_maps, list(range(NCORES)), trace=trace)
        LAST_RUN_INFO["exec_time_ns"] = res.exec_time_ns
        LAST_RUN_INFO["mean_exec_time_ns"] = res.mean_exec_time_ns
        LAST_RUN_INFO["profile_json"] = res.profile_json
        outs = [r["out"] for r in res.results]

    result = np.zeros((N, C), np.float32)
    for c in range(NCORES):
        y = np.asarray(outs[c]).astype(np.float32).reshape(2 * 128, NP)
        result[sids_per_core[c]] = y.T
    return result.reshape(B, N, C)


def _run_sim(nc, in_maps):
    from concourse import bass_interp

    outs = []
    for c in range(NCORES):
        sim = bass_interp.MultiCoreSim(nc, 1)
        for name, arr in in_maps[c].items():
            sim.cores[0].tensor(name)[:] = arr
        sim.simulate()
        LAST_RUN_INFO["sim_time_ns"] = sim.cores[0].time
        outs.append(np.array(sim.cores[0].tensor("out")))
        if os.environ.get("GRIDSTN_SIM_ONE_CORE"):
            outs = outs + [outs[0]] * (NCORES - 1)
            break
    return outs


# revision 8
# speedup vs baseline: 1.8675x; 1.2252x over previous
"""GridToStation Trainium2 kernel (bf16 patch-gather version).

Strategy:
  - Host: transpose grid (C,H,W) -> (H,W,C), cast bf16, and build per-core
    2x2 PATCH tables: row (y, xl) = [g[y,x], g[y,x1], g[y1,x], g[y1,x1]]
    (1024 bf16 = 2KB), where each core's table is a 212-column lon window
    around its 180-column band (x1/y1 border-clipped at build time).
  - Host: replicate the reference's f32 index math; sort stations by ix0 and
    cut every N/8 -> EXACTLY 2048 stations per core (fits each core's window
    because station lon is near-uniform; asserted). No padding anywhere.
  - Device (per core, SPMD over 8 cores), 16 tiles of 128 stations:
      * one indirect (gather) DMA per tile pulls 128 patch rows (2KB each).
        One descriptor per station is the minimum serial GpSimd/SWDGE work
        (~1.1us per call fixed) -- this is the kernel's critical path.
      * bilinear combine in [station, C] bf16: ACT seed (scale=c00) + 3 DVE
        scalar_tensor_tensor fused multiply-adds.
      * PE transpose (bf16) -> PSUM f32 -> ACT copy to bf16 x^T tiles.
      * MLP in [C, station] layout, groups of 512 stations: W1/W2 bf16
        matmuls (PSUM f32), GELU+b1 on ACT, b2 add on DVE, bf16 out.
      * per-group output DMA (overlapped), 2x[128,512] bf16 per group.
  - Host: upcast bf16 -> f32 and inverse-permute to original station order.
"""

import os

import numpy as np
import ml_dtypes

B, C, H, W, N = 1, 256, 721, 1440, 16384
NCORES = 8
COLS = W // NCORES  # 180 owned columns per core
FX = 16  # window flex columns each side
WT = COLS + 2 * FX  # 212-column table window
TROWS = H * WT  # patch-table rows per core
NP = N // NCORES  # 2048 stations per core, exact
T = NP // 128  # 16 tiles
GRP = 4  # tiles per MLP group (512 stations)
NG = T // GRP

_PROG_CACHE = {}


def _f32(x):
    return np.float32(x)


def _host_route(station_coords):
    """Replicate the reference index math in f32."""
    lat = np.asarray(station_coords[0, :, 0], dtype=np.float32)
    lon = np.asarray(station_coords[0, :, 1], dtype=np.float32)
    lat_n = lat / _f32(90.0)
    lon_n = lon / _f32(180.0)
    ix = np.clip((lon_n + _f32(1.0)) * _f32(0.5) * _f32(W - 1), _f32(0.0), _f32(W - 1))
    iy = np.clip((lat_n + _f32(1.0)) * _f32(0.5) * _f32(H - 1), _f32(0.0), _f32(H - 1))
    ix0f = np.floor(ix)
    iy0f = np.floor(iy)
    wx = (ix - ix0f).astype(np.float32)
    wy = (iy - iy0f).astype(np.float32)
    ix0 = ix0f.astype(np.int64)
    iy0 = iy0f.astype(np.int64)
    one = _f32(1.0)
    c00 = (one - wx) * (one - wy)
    c01 = wx * (one - wy)
    c10 = (one - wx) * wy
    c11 = wx * wy
    return ix0, iy0, (c00, c01, c10, c11)


def _host_tables(grid_features):
    """Global (H, W, C) bf16 grid + per-core patch-table windows."""
    g = np.asarray(grid_features[0], dtype=np.float32)  # (C, H, W)
    gt = np.transpose(g, (1, 2, 0)).astype(ml_dtypes.bfloat16)  # (H, W, C)
    # x+1 / y+1 with border clip
    gx1 = np.concatenate([gt[:, 1:, :], gt[:, W - 1 : W, :]], axis=1)
    gy1 = np.concatenate([gt[1:, :, :], gt[H - 1 : H, :, :]], axis=0)
    gx1y1 = np.concatenate([gy1[:, 1:, :], gy1[:, W - 1 : W, :]], axis=1)
    los = [min(max(c * COLS - FX, 0), W - WT) for c in range(NCORES)]
    tables = []
    for c in range(NCORES):
        lo = los[c]
        p = np.empty((H, WT, 4 * C), dtype=ml_dtypes.bfloat16)
        p[:, :, 0:C] = gt[:, lo : lo + WT]
        p[:, :, C : 2 * C] = gx1[:, lo : lo + WT]
        p[:, :, 2 * C : 3 * C] = gy1[:, lo : lo + WT]
        p[:, :, 3 * C : 4 * C] = gx1y1[:, lo : lo + WT]
        tables.append(p.reshape(TROWS, 4 * C))
    return tables, los


def _build_program():
    import concourse.bacc as bacc
    import concourse.bass as bass
    import concourse.mybir as mybir
    from concourse.tile import TileContext

    f32 = mybir.dt.float32
    bf16 = mybir.dt.bfloat16
    i32 = mybir.dt.int32

    nc = bacc.Bacc(
        "TRN2",
        target_bir_lowering=False,
        debug=False,
        dynamic_dma_scratch_size=49152,
    )

    tbl = nc.dram_tensor("tbl", [TROWS, 4 * C], bf16, kind="ExternalInput")
    idx = nc.dram_tensor("idx", [128, T], i32, kind="ExternalInput")
    cof = nc.dram_tensor("cof", [128, 4 * T], f32, kind="ExternalInput")
    w1 = nc.dram_tensor("w1t", [C, C], bf16, kind="ExternalInput")
    w2 = nc.dram_tensor("w2t", [C, C], bf16, kind="ExternalInput")
    bia = nc.dram_tensor("bia", [128, 4], f32, kind="ExternalInput")
    idn = nc.dram_tensor("idn", [128, 128], bf16, kind="ExternalInput")
    out = nc.dram_tensor("out", [2, 128, NP], bf16, kind="ExternalOutput")

    with TileContext(nc) as tc:
        with (
            tc.tile_pool(name="const", bufs=1) as cpool,
            tc.tile_pool(name="gat", bufs=1) as gpool,
            tc.tile_pool(name="sm", bufs=2 * GRP) as spool,
            tc.tile_pool(name="xs", bufs=2) as xpool,
            tc.tile_pool(name="hs", bufs=2) as hpool,
            tc.tile_pool(name="ys", bufs=2) as ypool,
            tc.tile_pool(name="px", bufs=2, space="PSUM") as pxp,
            tc.tile_pool(name="ph", bufs=1, space="PSUM") as php,
            tc.tile_pool(name="py", bufs=1, space="PSUM") as pyp,
        ):
            idx_sb = cpool.tile([128, T], i32)
            nc.sync.dma_start(out=idx_sb[:], in_=idx[:])
            cof_sb = cpool.tile([128, 4 * T], f32)
            nc.sync.dma_start(out=cof_sb[:], in_=cof[:])
            w1_sb = cpool.tile([128, 2 * C], bf16)
            nc.sync.dma_start(out=w1_sb[:, 0:C], in_=w1[0:128, :])
            nc.sync.dma_start(out=w1_sb[:, C : 2 * C], in_=w1[128:256, :])
            w2_sb = cpool.tile([128, 2 * C], bf16)
            nc.sync.dma_start(out=w2_sb[:, 0:C], in_=w2[0:128, :])
            nc.sync.dma_start(out=w2_sb[:, C : 2 * C], in_=w2[128:256, :])
            bia_sb = cpool.tile([128, 4], f32)
            nc.sync.dma_start(out=bia_sb[:], in_=bia[:])
            idn_sb = cpool.tile([128, 128], bf16)
            nc.sync.dma_start(out=idn_sb[:], in_=idn[:])

            # all gathers issued up-front; enough buffers that the GpSimd
            # queue never stalls (this is the serial critical path)
            gts = []
            for t in range(T):
                gt_t = gpool.tile([128, 1024], bf16, name=f"gt{t}")
                nc.gpsimd.indirect_dma_start(
                    out=gt_t[:],
                    out_offset=None,
                    in_=tbl[:],
                    in_offset=bass.IndirectOffsetOnAxis(
                        ap=idx_sb[:, t : t + 1], axis=0
                    ),
                )
                gts.append(gt_t)

            for g in range(NG):
                pxs = [pxp.tile([128, 512], bf16, name=f"px{c}") for c in range(2)]
                for tt in range(GRP):
                    t = g * GRP + tt
                    gt_t = gts[t]
                    sm = spool.tile([128, 256], bf16, name="sm")
                    for j in range(4):
                        vj = gt_t[:, j * 256 : (j + 1) * 256]
                        cj = cof_sb[:, j * T + t : j * T + t + 1]
                        if j == 0:
                            nc.vector.tensor_scalar_mul(sm[:], vj, cj)
                        else:
                            nc.vector.scalar_tensor_tensor(
                                out=sm[:],
                                in0=vj,
                                scalar=cj,
                                in1=sm[:],
                                op0=mybir.AluOpType.mult,
                                op1=mybir.AluOpType.add,
                            )
                    for ch in range(2):
                        nc.tensor.transpose(
                            out=pxs[ch][:, tt * 128 : (tt + 1) * 128],
                            in_=sm[:, ch * 128 : (ch + 1) * 128],
                            identity=idn_sb[:],
                        )
                xss = [xpool.tile([128, 512], bf16, name=f"xs{c}") for c in range(2)]
                for ch in range(2):
                    nc.scalar.activation(
                        out=xss[ch][:],
                        in_=pxs[ch][:],
                        func=mybir.ActivationFunctionType.Copy,
                    )
                ph = php.tile([128, 1024], f32, name="ph")
                for m in range(2):
                    for k in range(2):
                        nc.tensor.matmul(
                            out=ph[:, m * 512 : (m + 1) * 512],
                            lhsT=w1_sb[:, k * C + m * 128 : k * C + (m + 1) * 128],
                            rhs=xss[k][:],
                            start=(k == 0),
                            stop=(k == 1),
                        )
                hss = [hpool.tile([128, 512], bf16, name=f"hs{c}") for c in range(2)]
                for m in range(2):
                    nc.scalar.activation(
                        out=hss[m][:],
                        in_=ph[:, m * 512 : (m + 1) * 512],
                        func=mybir.ActivationFunctionType.Gelu,
                        bias=bia_sb[:, m : m + 1],
                        scale=1.0,
                    )
                py = pyp.tile([128, 1024], f32, name="py")
                for m in range(2):
                    for k in range(2):
                        nc.tensor.matmul(
                            out=py[:, m * 512 : (m + 1) * 512],
                            lhsT=w2_sb[:, k * C + m * 128 : k * C + (m + 1) * 128],
                            rhs=hss[k][:],
                            start=(k == 0),
                            stop=(k == 1),
                        )
                yss = [ypool.tile([128, 512], bf16, name=f"ys{c}") for c in range(2)]
                for m in range(2):
                    nc.scalar.activation(
                        out=yss[m][:],
                        in_=py[:, m * 512 : (m + 1) * 512],
                        func=mybir.ActivationFunctionType.Identity,
                        bias=bia_sb[:, 2 + m : 3 + m],
                        scale=1.0,
                    )
                    nc.sync.dma_start(
                        out=out[m][:, g * 512 : (g + 1) * 512], in_=yss[m][:]
                    )
    return nc


def _make_in_maps(grid_features, station_coords, W1, b1, W2, b2):
    ix0, iy0, cjs = _host_route(station_coords)
    tables, los = _host_tables(grid_features)

    order = np.argsort(ix0, kind="stable")
    w1t = np.ascontiguousarray(np.asarray(W1, np.float32).T).astype(ml_dtypes.bfloat16)
    w2t = np.ascontiguousarray(np.asarray(W2, np.float32).T).astype(ml_dtypes.bfloat16)
    bia = np.zeros((128, 4), np.float32)
    bia[:, 0] = b1[0:128]
    bia[:, 1] = b1[128:256]
    bia[:, 2] = b2[0:128]
    bia[:, 3] = b2[128:256]
    idn = np.eye(128, dtype=ml_dtypes.bfloat16)

    in_maps = []
    sids_per_core = []
    for c in range(NCORES):
        sids = order[c * NP : (c + 1) * NP]
        sids_per_core.append(sids)
        xl = ix0[sids] - los[c]
        assert xl.min() >= 0 and xl.max() < WT, (
            f"core {c}: station lon outside table window "
            f"({xl.min()}..{xl.max()} vs 0..{WT - 1})"
        )
        rows = (iy0[sids] * WT + xl).astype(np.int32)
        idx_arr = np.ascontiguousarray(rows.reshape(T, 128).T)  # [128, T]
        cof_arr = np.ascontiguousarray(
            np.concatenate(
                [cjs[j][sids].reshape(T, 128).T for j in range(4)], axis=1
            )
        ).astype(np.float32)
        in_maps.append(
            {
                "tbl": tables[c],
                "idx": idx_arr,
                "cof": cof_arr,
                "w1t": w1t,
                "w2t": w2t,
                "bia": bia,
                "idn": idn,
            }
        )
    return in_maps, sids_per_core


LAST_RUN_INFO = {}


def _install_ntff_shim():
    """This container's antenv lacks axon_hooks; provide the NTFF profile
    hook via the same ctypes path trn_boot would have used."""
    import sys
    import types

    try:
        import antenv.axon_hooks  # noqa: F401

        return
    except ImportError:
        pass
    from trn_agent_boot.trn_boot import _ntff_profile_via_ctypes

    hook = _ntff_profile_via_ctypes("/opt/axon/libaxon_pjrt.so")
    mod = types.ModuleType("antenv.axon_hooks")
    mod.get_axon_ntff_profile_hook = lambda: hook
    mod.set_axon_ntff_profile_hook = lambda h: None
    sys.modules["antenv.axon_hooks"] = mod


def kernel(grid_features, station_coords, W1, b1, W2, b2):
    in_maps, sids_per_core = _make_in_maps(
        grid_features, station_coords, W1, b1, W2, b2
    )

    if "prog" not in _PROG_CACHE:
        _PROG_CACHE["prog"] = _build_program()
    nc = _PROG_CACHE["prog"]

    if os.environ.get("GRIDSTN_SIM"):
        outs = _run_sim(nc, in_maps)
    else:
        from concourse.bass_utils import run_bass_kernel_spmd

        trace = bool(os.environ.get("GRIDSTN_TRACE"))
        if trace:
            _install_ntff_shim()
        if not nc.is_finalized():
            nc.finalize()
        res = run_bass_kernel_spmd(nc, in_maps, list(range(NCORES)), trace=trace)
        LAST_RUN_INFO["exec_time_ns"] = res.exec_time_ns
        LAST_RUN_INFO["mean_exec_time_ns"] = res.mean_exec_time_ns
        LAST_RUN_INFO["profile_json"] = res.profile_json
        outs = [r["out"] for r in res.results]

    result = np.zeros((N, C), np.float32)
    for c in range(NCORES):
        y = np.asarray(outs[c]).astype(np.float32).reshape(2 * 128, NP)
        result[sids_per_core[c]] = y.T
    return result.reshape(B, N, C)


def _run_sim(nc, in_maps):
    from concourse import bass_interp

    outs = []
    for c in range(NCORES):
        sim = bass_interp.MultiCoreSim(nc, 1)
        for name, arr in in_maps[c].items():
            sim.cores[0].tensor(name)[:] = arr
        sim.simulate()
        LAST_RUN_INFO["sim_time_ns"] = sim.cores[0].time
        outs.append(np.array(sim.cores[0].tensor("out")))
        if os.environ.get("GRIDSTN_SIM_ONE_CORE"):
            outs = outs + [outs[0]] * (NCORES - 1)
            break
    return outs
